# revision 1
# baseline (speedup 1.0000x reference)
"""BertTinyFlatten on 8 Trainium2 NeuronCores — data-parallel over batch.

Per core (one batch element):
  emb   = gather(word_emb, x) + (pos_emb + tok_emb[0])      [indirect DMA w/ CCE add]
  x0    = layernorm(emb)                                     [token-major, DVE/ACT]
  x0t   = x0.T (PE transpose, LN affine fused into PSUM->SBUF copy)
  y1    = x0 @ init_d.T          (token-major out)           [f32r matmuls]
  y1sq  = (mix(y1, init_M) + b1)^2    -> feature-major       [ACT Square fused]
  y2    = y1sq-chain @ inter0_d.T     -> token-major
  y2sq  = (mix(y2, inter0_M) + b2)^2  -> feature-major
  yt    = final_d-chain @ y2sq + b3   -> feature-major; host transposes back

All matmuls run as float32r (FP32 truncated to ~FP22 inside the PE) at
bf16 rate: 1 cycle/row for N=512.
"""
import os
import sys

import numpy as np
import ml_dtypes

for _p in ("/opt/trn_rl_repo", "/opt/pypackages"):
    if _p not in sys.path and os.path.isdir(_p):
        sys.path.append(_p)

from contextlib import ExitStack

import concourse.bass as bass
import concourse.tile as tile
from concourse import bacc, masks, mybir
from concourse.bass import IndirectOffsetOnAxis
from concourse.bass_utils import run_bass_kernel_spmd

f32 = mybir.dt.float32
f32r = mybir.dt.float32r
bf16 = mybir.dt.bfloat16
i32 = mybir.dt.int32
AF = mybir.ActivationFunctionType
ALU = mybir.AluOpType
AX = mybir.AxisListType

B, S, HID, NH, INTER, VOCAB = 8, 1024, 512, 8, 2048, 30522
DH = INTER // NH            # 256 features per head
EPS = 1e-12
N_CORES = 8

KH = HID // 128             # 4   k-tiles for dense1
KI = INTER // 128           # 16  k-tiles for dense2/3
SC = S // 128               # 8   token chunks
NC1 = INTER // 512          # 4   n-chunks (512) for dense1/2
HT = HID // 128             # 4   hid tiles for dense3

STAGES = ("A", "B", "C", "D", "E", "full")


def _build_program(stage="full"):
    upto = STAGES.index(stage)
    nc = bacc.Bacc("TRN2", target_bir_lowering=False, debug=False,
                   num_devices=N_CORES, num_swdge_queues=2)

    xw = nc.dram_tensor("xw", [128, SC], i32, kind="ExternalInput").ap()
    word_emb = nc.dram_tensor("word_emb", [VOCAB, HID], f32, kind="ExternalInput").ap()
    posplus = nc.dram_tensor("posplus", [S, HID], f32, kind="ExternalInput").ap()
    lnw = nc.dram_tensor("lnw", [128, HT], f32, kind="ExternalInput").ap()
    lnb = nc.dram_tensor("lnb", [128, HT], f32, kind="ExternalInput").ap()
    w1t = nc.dram_tensor("w1t", [HID, INTER], f32, kind="ExternalInput").ap()
    b1c = nc.dram_tensor("b1c", [128, KI], f32, kind="ExternalInput").ap()
    m1 = nc.dram_tensor("m1", [NH, S, S], bf16, kind="ExternalInput").ap()
    w2t = nc.dram_tensor("w2t", [INTER, INTER], f32, kind="ExternalInput").ap()
    b2c = nc.dram_tensor("b2c", [128, KI], f32, kind="ExternalInput").ap()
    m2 = nc.dram_tensor("m2", [NH, S, S], bf16, kind="ExternalInput").ap()
    w3t = nc.dram_tensor("w3t", [INTER, HID], f32, kind="ExternalInput").ap()
    b3c = nc.dram_tensor("b3c", [128, HT], f32, kind="ExternalInput").ap()
    yt_out = nc.dram_tensor("yt", [HID, S], f32, kind="ExternalOutput").ap()

    with tile.TileContext(nc) as tc, ExitStack() as ctx:
        pool = ctx.enter_context(tc.tile_pool(name="sbuf", bufs=1))
        psum = ctx.enter_context(tc.tile_pool(name="psum", bufs=1, space="PSUM"))

        def dump(tiles):
            # debug: write four [128, >=S] tiles to yt_out
            for i, t in enumerate(tiles[:4]):
                nc.sync.dma_start(yt_out[i * 128:(i + 1) * 128, :],
                                  t[:, 0:S].bitcast(f32))

        # ---- token ids first: the gathers depend only on this ----------
        t_idx = pool.tile([128, SC], i32)
        nc.sync.dma_start(t_idx[:], xw[:])

        # ---- posplus chunks, then gather-with-add on top ---------------
        emb = []
        for c in range(SC):
            e = pool.tile([128, HID], f32, tag="emb", bufs=SC, name=f"emb{c}")
            nc.sync.dma_start(e[:], posplus[c * 128:(c + 1) * 128, :])
            emb.append(e)
        for c in range(SC):
            gi = nc.gpsimd.indirect_dma_start(
                out=emb[c][:], out_offset=None,
                in_=word_emb[:128, :],
                in_offset=IndirectOffsetOnAxis(ap=t_idx[:, c:c + 1], axis=0),
                bounds_check=VOCAB - 1, oob_is_err=False,
                compute_op=ALU.add,
            )
            if c % 2:
                gi.ins.queue = "qPoolDynamic1"

        # ---- constants -------------------------------------------------
        ident = pool.tile([128, 128], f32)
        masks.make_identity(nc, ident[:])
        zerocol = pool.tile([128, 1], f32)
        nc.vector.memset(zerocol[:], 0.0)
        epscol = pool.tile([128, 1], f32)
        nc.vector.memset(epscol[:], EPS)
        t_lnw = pool.tile([128, HT], f32)
        nc.sync.dma_start(t_lnw[:], lnw[:])
        t_lnb = pool.tile([128, HT], f32)
        nc.sync.dma_start(t_lnb[:], lnb[:])
        t_b1 = pool.tile([128, KI], f32)
        nc.sync.dma_start(t_b1[:], b1c[:])
        t_b2 = pool.tile([128, KI], f32)
        nc.sync.dma_start(t_b2[:], b2c[:])
        t_b3 = pool.tile([128, HT], f32)
        nc.sync.dma_start(t_b3[:], b3c[:])
        # ---- stage A: per-chunk layernorm (invoked from fused dense1) --
        def layernorm_chunk(c):
            e = emb[c]
            msum = pool.tile([128, 1], f32, tag="msum", bufs=2, name=f"msum{c}")
            nc.vector.reduce_sum(msum[:], e[:], axis=AX.X)
            sqd = pool.tile([128, HID], f32, tag="sqd", bufs=2, name=f"sqd{c}")
            ssq = pool.tile([128, 1], f32, tag="ssq", bufs=2, name=f"ssq{c}")
            nc.scalar.activation(sqd[:], e[:], AF.Square, bias=zerocol[:],
                                 accum_out=ssq[:])
            # var = ssq/H - (msum/H)^2 ; rstd = 1/sqrt(var+eps)
            t1 = pool.tile([128, 1], f32, tag="t1", bufs=2, name=f"t1_{c}")
            nc.vector.tensor_scalar(t1[:], msum[:], msum[:], 1.0 / (HID * HID),
                                    op0=ALU.mult, op1=ALU.mult)
            var = pool.tile([128, 1], f32, tag="var", bufs=2, name=f"var{c}")
            nc.vector.tensor_scalar(var[:], ssq[:], 1.0 / HID, t1[:],
                                    op0=ALU.mult, op1=ALU.subtract)
            std = pool.tile([128, 1], f32, tag="std", bufs=2, name=f"std{c}")
            nc.scalar.activation(std[:], var[:], AF.Sqrt, bias=epscol[:])
            rstd = pool.tile([128, 1], f32, tag="rstd", bufs=2, name=f"rstd{c}")
            nc.vector.reciprocal(rstd[:], std[:])
            # e = e*rstd - (msum/H)*rstd
            mr = pool.tile([128, 1], f32, tag="mr", bufs=2, name=f"mr{c}")
            nc.vector.tensor_scalar(mr[:], msum[:], rstd[:], 1.0 / HID,
                                    op0=ALU.mult, op1=ALU.mult)
            nc.vector.tensor_scalar(e[:], e[:], rstd[:], mr[:],
                                    op0=ALU.mult, op1=ALU.subtract)

        # feature-major activations live in the 16-slot "featmaj" ring:
        # x0t (4 tiles) -> y1sq (16) -> y2sq (16), WAR-serialized by Tile.
        x0t = []
        for ht in range(HT):
            x0t.append(pool.tile([128, S], f32r, tag="featmaj", bufs=16,
                                 name=f"x0t{ht}"))

        def mix(yin, m_ap, bias_tile, out_name):
            # per-head seq mix + bias + square; token-major in, feature-major out
            ysq = []
            for h in range(NH):
                groups = [[None] * 2 for _ in range(2)]
                for tc_i in range(2):
                    for dp in range(2):
                        groups[tc_i][dp] = psum.tile(
                            [128, 512], f32, tag="mm", bufs=8,
                            name=f"{out_name}p{h}_{tc_i}_{dp}")
                for s in range(SC):
                    mt = pool.tile([128, S], bf16, tag="mring", bufs=8,
                                   name=f"{out_name}m{h}_{s}")
                    nc.sync.dma_start(mt[:], m_ap[h, s * 128:(s + 1) * 128, :])
                    for dp in range(2):
                        lhsT = yin[s][:, h * DH + dp * 128: h * DH + (dp + 1) * 128]
                        for tc_i in range(2):
                            nc.tensor.matmul(groups[tc_i][dp][:], lhsT,
                                             mt[:, tc_i * 512:(tc_i + 1) * 512],
                                             start=(s == 0), stop=(s == SC - 1))
                for dp in range(2):
                    i = h * 2 + dp
                    yo = pool.tile([128, S], f32r, tag="featmaj", bufs=16,
                                   name=f"{out_name}{i}")
                    for tc_i in range(2):
                        nc.scalar.activation(yo[:, tc_i * 512:(tc_i + 1) * 512],
                                             groups[tc_i][dp][:], AF.Square,
                                             bias=bias_tile[:, i:i + 1])
                    ysq.append(yo)
            return ysq

        def dense(xin, w_ap, nk, nm, transpose_src=None):
            # token-major out: y[s, n] = x @ w  (xin: feature-major tiles).
            # With transpose_src, loop s-major and emit the PE transposes that
            # produce xin[:, s] right before the s-group (stage A/B fusion).
            yt = []
            for s in range(SC):
                yt.append(pool.tile([128, INTER], bf16, tag="tokmaj", bufs=SC,
                                    name=f"{nm}{s}"))
            if transpose_src is None:
                for n in range(NC1):
                    wts = []
                    for k in range(nk):
                        wt = pool.tile([128, 512], f32r, tag="wring", bufs=16,
                                       name=f"{nm}w{n}_{k}")
                        nc.sync.dma_start(wt[:], w_ap[k * 128:(k + 1) * 128,
                                                      n * 512:(n + 1) * 512]
                                          .bitcast(f32r))
                        wts.append(wt)
                    for s in range(SC):
                        ps = psum.tile([128, 512], f32, tag="mm", bufs=8,
                                       name=f"{nm}p{n}_{s}")
                        for k in range(nk):
                            nc.tensor.matmul(ps[:], xin[k][:, s * 128:(s + 1) * 128],
                                             wts[k][:], start=(k == 0),
                                             stop=(k == nk - 1))
                        nc.scalar.copy(yt[s][:, n * 512:(n + 1) * 512], ps[:])
                return yt
            # fused: preload all nk*NC1 weight tiles, then s-major
            wts = {}
            for n in range(NC1):
                for k in range(nk):
                    wt = pool.tile([128, 512], f32r, tag="wring", bufs=16,
                                   name=f"{nm}w{n}_{k}")
                    nc.sync.dma_start(wt[:], w_ap[k * 128:(k + 1) * 128,
                                                  n * 512:(n + 1) * 512]
                                      .bitcast(f32r))
                    wts[(n, k)] = wt
            for s in range(SC):
                layernorm_chunk(s)
                for ht in range(HT):
                    pt = psum.tile([128, 512], f32, tag="mm", bufs=8,
                                   name=f"ptr{ht}_{s}")
                    nc.tensor.transpose(pt[:, 0:128],
                                        transpose_src[s][:, ht * 128:(ht + 1) * 128],
                                        ident[:])
                    nc.vector.tensor_scalar(
                        xin[ht][:, s * 128:(s + 1) * 128], pt[:, 0:128],
                        t_lnw[:, ht:ht + 1], t_lnb[:, ht:ht + 1],
                        op0=ALU.mult, op1=ALU.add)
                for n in range(NC1):
                    ps = psum.tile([128, 512], f32, tag="mm", bufs=8,
                                   name=f"{nm}p{n}_{s}")
                    for k in range(nk):
                        nc.tensor.matmul(ps[:], xin[k][:, s * 128:(s + 1) * 128],
                                         wts[(n, k)][:], start=(k == 0),
                                         stop=(k == nk - 1))
                    nc.scalar.copy(yt[s][:, n * 512:(n + 1) * 512], ps[:])
            return yt

        if upto == 0:                       # stage A only
            dump(x0t)
        if upto >= 1:
            y1 = dense(x0t, w1t, KH, "y1_", transpose_src=emb)
            if upto == 1:
                dump(y1)
        if upto >= 2:
            y1sq = mix(y1, m1, t_b1, "y1sq")
            if upto == 2:
                dump(y1sq)
        if upto >= 3:
            y2 = dense(y1sq, w2t, KI, "y2_")
            if upto == 3:
                dump(y2)
        if upto >= 4:
            y2sq = mix(y2, m2, t_b2, "y2sq")
            if upto == 4:
                dump(y2sq)
        if upto >= 5:                       # dense3 + bias + store
            w3ts = []
            for k in range(KI):
                w3tile = pool.tile([128, 512], f32r, tag="wring", bufs=16,
                                   name=f"w3_{k}")
                nc.sync.dma_start(w3tile[:],
                                  w3t[k * 128:(k + 1) * 128, :].bitcast(f32r))
                w3ts.append(w3tile)
            for ht in range(HT):
                yo = pool.tile([128, S], f32, tag="out", bufs=2, name=f"yt{ht}")
                pss = [psum.tile([128, 512], f32, tag="mm", bufs=8,
                                 name=f"p3_{ht}_{sc}") for sc in range(2)]
                for k in range(KI):
                    for sc in range(2):
                        nc.tensor.matmul(pss[sc][:],
                                         w3ts[k][:, ht * 128:(ht + 1) * 128],
                                         y2sq[k][:, sc * 512:(sc + 1) * 512],
                                         start=(k == 0), stop=(k == KI - 1))
                for sc in range(2):
                    nc.scalar.activation(yo[:, sc * 512:(sc + 1) * 512], pss[sc][:],
                                         AF.Identity, bias=t_b3[:, ht:ht + 1])
                    nc.sync.dma_start(
                        yt_out[ht * 128:(ht + 1) * 128, sc * 512:(sc + 1) * 512],
                        yo[:, sc * 512:(sc + 1) * 512])

    nc.compile()
    return nc


_PROGRAMS = {}
LAST_RESULT = None


def _get_program(stage="full"):
    if stage not in _PROGRAMS:
        _PROGRAMS[stage] = _build_program(stage)
    return _PROGRAMS[stage]


def _prep_maps(x, word_emb, pos_emb, tok_emb, emb_ln_w, emb_ln_b,
               init_d, init_b, init_M, inter0_d, inter0_b, inter0_M,
               final_d, final_b):
    x = np.asarray(x)
    f = lambda a: np.ascontiguousarray(np.asarray(a), dtype=np.float32)
    shared = dict(
        word_emb=f(word_emb),
        posplus=f(pos_emb) + f(tok_emb)[0][None, :],
        lnw=np.ascontiguousarray(f(emb_ln_w).reshape(HT, 128).T),
        lnb=np.ascontiguousarray(f(emb_ln_b).reshape(HT, 128).T),
        w1t=np.ascontiguousarray(f(init_d).T),
        b1c=np.ascontiguousarray(f(init_b).reshape(KI, 128).T),
        m1=np.ascontiguousarray(np.asarray(init_M)).astype(ml_dtypes.bfloat16),
        w2t=np.ascontiguousarray(f(inter0_d).T),
        b2c=np.ascontiguousarray(f(inter0_b).reshape(KI, 128).T),
        m2=np.ascontiguousarray(np.asarray(inter0_M)).astype(ml_dtypes.bfloat16),
        w3t=np.ascontiguousarray(f(final_d).T),
        b3c=np.ascontiguousarray(f(final_b).reshape(HT, 128).T),
    )
    in_maps = []
    for b in range(B):
        xwb = np.ascontiguousarray(x[b].astype(np.int32).reshape(SC, 128).T)
        in_maps.append(dict(shared, xw=xwb))
    return in_maps


def kernel(**inputs):
    global LAST_RESULT
    stage = os.environ.get("KSTAGE", "full")
    ncores = int(os.environ.get("KCORES", str(N_CORES)))
    in_maps = _prep_maps(**inputs)[:ncores]
    nc = _get_program(stage)
    res = run_bass_kernel_spmd(nc, in_maps, list(range(ncores)))
    LAST_RESULT = res
    out = np.stack([res.results[b]["yt"].T for b in range(ncores)])
    if ncores < B:
        out = np.concatenate([out] + [out[:1]] * (B - ncores))
    return out



# revision 10
# speedup vs baseline: 1.0702x; 1.0702x over previous
"""BertTinyFlatten on 8 Trainium2 NeuronCores — data-parallel over batch.

Per core (one batch element):
  emb   = gather(word_emb, x) + (pos_emb + tok_emb[0])      [bf16, indirect DMA w/ CCE add]
  x0    = layernorm(emb)                                     [bf16 out]
  x0t   = x0.T (PE transpose bf16, LN affine fused into PSUM->SBUF copy)
  y1q   = fp8e4(x0 @ w1_scaled)  (k-outer bf16 matmuls; per-feature scales
          folded into w1 columns host-side; ACT copy casts PSUM->fp8)
  y1sq  = (mix_fp8dr(y1q, M1q) * c + b1)^2 -> feature-major  [fp8 DoubleRow
          matmuls at 2x rate; scales unfolded via ACT Square scale/bias]
  y2    = y1sq-chain @ inter0_d.T     -> token-major bf16    [f32r matmuls]
  y2sq  = (mix(y2, inter0_M) + b2)^2  -> feature-major       [bf16 matmuls]
  yt    = final_d-chain @ y2sq + b3   -> feature-major; host transposes back

DMA plan: w1 halfslabs interleaved with posplus chunks on the sync queue so
dense1 can start ~13us; gathers (software DGE) issue immediately after t_idx.
"""
import os
import sys

import numpy as np
import ml_dtypes

for _p in ("/opt/trn_rl_repo", "/opt/pypackages"):
    if _p not in sys.path and os.path.isdir(_p):
        sys.path.append(_p)

from contextlib import ExitStack

import concourse.bass as bass
import concourse.tile as tile
from concourse import bacc, masks, mybir
from concourse.bass import IndirectOffsetOnAxis
from concourse.bass_utils import run_bass_kernel_spmd

f32 = mybir.dt.float32
f32r = mybir.dt.float32r
bf16 = mybir.dt.bfloat16
f8 = mybir.dt.float8e4
i32 = mybir.dt.int32
AF = mybir.ActivationFunctionType
ALU = mybir.AluOpType
AX = mybir.AxisListType
PM = mybir.MatmulPerfMode

B, S, HID, NH, INTER, VOCAB = 8, 1024, 512, 8, 2048, 30522
DH = INTER // NH            # 256 features per head
EPS = 1e-12
N_CORES = 8

KH = HID // 128             # 4   k-tiles for dense1
KI = INTER // 128           # 16  k-tiles for dense2/3
SC = S // 128               # 8   token chunks
NC1 = INTER // 512          # 4   n-chunks (512) for dense1/2
HT = HID // 128             # 4   hid tiles for dense3
SB = S // 256               # 4   256-deep s-blocks for fp8 DoubleRow mix

STAGES = ("C", "D", "E", "full")


def _build_program(stage="full"):
    upto = STAGES.index(stage)
    nc = bacc.Bacc("TRN2", target_bir_lowering=False, debug=False,
                   num_devices=N_CORES, num_swdge_queues=2)

    xw = nc.dram_tensor("xw", [128, SC], i32, kind="ExternalInput").ap()
    word_emb = nc.dram_tensor("word_emb", [VOCAB, HID], f32, kind="ExternalInput").ap()
    posplus = nc.dram_tensor("posplus", [S, HID], f32, kind="ExternalInput").ap()
    lnw = nc.dram_tensor("lnw", [128, HT], f32, kind="ExternalInput").ap()
    lnb = nc.dram_tensor("lnb", [128, HT], f32, kind="ExternalInput").ap()
    w1t = nc.dram_tensor("w1t", [HID, INTER], bf16, kind="ExternalInput").ap()
    b1c = nc.dram_tensor("b1c", [128, KI], f32, kind="ExternalInput").ap()
    c1c = nc.dram_tensor("c1c", [128, KI], f32, kind="ExternalInput").ap()
    m1f8 = nc.dram_tensor("m1f8", [NH, SB, 128, 2, S], f8, kind="ExternalInput").ap()
    w2t = nc.dram_tensor("w2t", [INTER, INTER], f32, kind="ExternalInput").ap()
    b2c = nc.dram_tensor("b2c", [128, KI], f32, kind="ExternalInput").ap()
    m2 = nc.dram_tensor("m2", [NH, S, S], bf16, kind="ExternalInput").ap()
    w3t = nc.dram_tensor("w3t", [INTER, HID], f32, kind="ExternalInput").ap()
    b3c = nc.dram_tensor("b3c", [128, HT], f32, kind="ExternalInput").ap()
    yt_out = nc.dram_tensor("yt", [HID, S], f32, kind="ExternalOutput").ap()

    with tile.TileContext(nc) as tc, ExitStack() as ctx:
        pool = ctx.enter_context(tc.tile_pool(name="sbuf", bufs=1))
        psum = ctx.enter_context(tc.tile_pool(name="psum", bufs=1, space="PSUM"))

        def dump(tiles):
            # debug: write up to four [128, >=S]-byte tiles (as f32 words)
            for i, t in enumerate(tiles[:4]):
                v = t[:].bitcast(f32)
                w = min(v.shape[-1], S)
                nc.sync.dma_start(yt_out[i * 128:(i + 1) * 128, 0:w], v[:, 0:w])

        # ---- token ids first: the gathers depend only on this ----------
        t_idx = pool.tile([128, SC], i32)
        nc.sync.dma_start(t_idx[:], xw[:])

        # ---- constants needed by the PE early: emit on gpsimd BEFORE the
        # serialized gather instructions.
        ident = pool.tile([128, 128], f32)
        masks.make_identity(nc, ident[:])
        zerocol = pool.tile([128, 1], f32)
        nc.vector.memset(zerocol[:], 0.0)

        # ---- posplus chunks interleaved with w1 halfslabs on sync:
        # chunk c first so gather c can start, w1 k-slabs in PE use order.
        emb = []
        w1hs = []
        for c in range(SC):
            e = pool.tile([128, HID], f32, tag="emb", bufs=SC, name=f"emb{c}")
            nc.sync.dma_start(e[:], posplus[c * 128:(c + 1) * 128, :])
            emb.append(e)
            for j in ([0, 1] if c == 0 else [c * 2, c * 2 + 1]) if c < KH else []:
                w = pool.tile([128, 1024], bf16, tag="wring", bufs=16,
                              name=f"w1hs{j}")
                nc.sync.dma_start(w[:], w1t[(j // 2) * 128:(j // 2 + 1) * 128,
                                            (j % 2) * 1024:(j % 2 + 1) * 1024])
                w1hs.append(w)

        # ---- indirect gathers add word_emb rows onto posplus chunks ----
        for c in range(SC):
            gi = nc.gpsimd.indirect_dma_start(
                out=emb[c][:], out_offset=None,
                in_=word_emb[:128, :],
                in_offset=IndirectOffsetOnAxis(ap=t_idx[:, c:c + 1], axis=0),
                bounds_check=VOCAB - 1, oob_is_err=False,
                compute_op=ALU.add,
            )
            if c % 2:
                gi.ins.queue = "qPoolDynamic1"

        # ---- remaining constants ---------------------------------------
        t_lnw = pool.tile([128, HT], f32)
        nc.sync.dma_start(t_lnw[:], lnw[:])
        t_lnb = pool.tile([128, HT], f32)
        nc.sync.dma_start(t_lnb[:], lnb[:])
        t_b1 = pool.tile([128, KI], f32)
        nc.sync.dma_start(t_b1[:], b1c[:])
        t_c1 = pool.tile([128, KI], f32)
        nc.sync.dma_start(t_c1[:], c1c[:])
        t_b2 = pool.tile([128, KI], f32)
        nc.sync.dma_start(t_b2[:], b2c[:])
        t_b3 = pool.tile([128, HT], f32)
        nc.sync.dma_start(t_b3[:], b3c[:])

        def layernorm_chunk(c):
            # all-DVE except the Square+accum (Scalar, runs concurrently):
            # rstd via fast-inverse-sqrt (2 Newton iters, rel err < 5e-6)
            # to avoid Vector<->Scalar semaphore round trips on the head
            # critical path.
            e = emb[c]
            msum = pool.tile([128, 1], f32, tag="msum", bufs=2, name=f"msum{c}")
            nc.vector.reduce_sum(msum[:], e[:], axis=AX.X)
            sqd = pool.tile([128, HID], f32, tag="sqd", bufs=2, name=f"sqd{c}")
            ssq = pool.tile([128, 1], f32, tag="ssq", bufs=2, name=f"ssq{c}")
            nc.scalar.activation(sqd[:], e[:], AF.Square, bias=zerocol[:],
                                 accum_out=ssq[:])
            # ve = var + eps = ssq/H - (msum^2/H^2 - eps)
            t1 = pool.tile([128, 1], f32, tag="t1", bufs=2, name=f"t1_{c}")
            nc.vector.tensor_scalar(t1[:], msum[:], msum[:], 1.0 / (HID * HID),
                                    op0=ALU.mult, op1=ALU.mult)
            nc.vector.tensor_scalar(t1[:], t1[:], EPS, None, op0=ALU.subtract)
            ve = pool.tile([128, 1], f32, tag="var", bufs=2, name=f"var{c}")
            nc.vector.tensor_scalar(ve[:], ssq[:], 1.0 / HID, t1[:],
                                    op0=ALU.mult, op1=ALU.subtract)
            # r0 = bitcast(0x5F3759DF - (bitcast_i32(ve) >> 1))
            rstd = pool.tile([128, 1], f32, tag="rstd", bufs=2, name=f"rstd{c}")
            ri = rstd[:].bitcast(i32)
            nc.vector.tensor_scalar(ri, ve[:].bitcast(i32), 1, None,
                                    op0=ALU.arith_shift_right)
            nc.vector.tensor_scalar(ri, ri, -1, 0x5F3759DF,
                                    op0=ALU.mult, op1=ALU.add)
            # Newton x2: r = r * (1.5 - 0.5*ve*r^2)
            nt = pool.tile([128, 1], f32, tag="nt", bufs=2, name=f"nt{c}")
            for _ in range(2):
                nc.vector.tensor_scalar(nt[:], rstd[:], rstd[:], ve[:],
                                        op0=ALU.mult, op1=ALU.mult)
                nc.vector.tensor_scalar(nt[:], nt[:], -0.5, 1.5,
                                        op0=ALU.mult, op1=ALU.add)
                nc.vector.tensor_scalar(rstd[:], rstd[:], nt[:], None,
                                        op0=ALU.mult)
            # e = e*rstd - (msum/H)*rstd
            mr = pool.tile([128, 1], f32, tag="mr", bufs=2, name=f"mr{c}")
            nc.vector.tensor_scalar(mr[:], msum[:], rstd[:], 1.0 / HID,
                                    op0=ALU.mult, op1=ALU.mult)
            nc.vector.tensor_scalar(e[:], e[:], rstd[:], mr[:],
                                    op0=ALU.mult, op1=ALU.subtract)

        # feature-major f32r ring: x0t slots (bitcast bf16) -> y1sq (16)
        # -> y2sq (16), WAR-serialized by Tile.
        x0t = []
        for ht in range(HT):
            slot = pool.tile([128, S], f32r, tag="featmaj", bufs=16,
                             name=f"x0t{ht}")
            x0t.append(slot[:].bitcast(bf16))           # [128, 2S] bf16 view

        # y1 in fp8, laid out [128 tok-in-chunk, SC, INTER]
        y1q = pool.tile([128, SC, INTER], f8, name="y1q")

        def dense1():
            # k-outer so early w1 slabs are consumed as they arrive
            for s in range(SC):
                layernorm_chunk(s)
                for ht in range(HT):
                    pt = psum.tile([128, 512], f32, tag="mm", bufs=8,
                                   name=f"ptr{ht}_{s}")
                    nc.tensor.transpose(pt[:, 0:128],
                                        emb[s][:, ht * 128:(ht + 1) * 128],
                                        ident[:])
                    nc.scalar.activation(
                        x0t[ht][:, s * 128:(s + 1) * 128], pt[:, 0:128],
                        AF.Identity, bias=t_lnb[:, ht:ht + 1],
                        scale=t_lnw[:, ht:ht + 1])
                ps = [psum.tile([128, 512], f32, tag="mm", bufs=8,
                                name=f"d1p{n}_{s}") for n in range(NC1)]
                for k in range(KH):
                    for n in range(NC1):
                        nc.tensor.matmul(ps[n][:],
                                         x0t[k][:, s * 128:(s + 1) * 128],
                                         w1hs[k * 2 + n // 2][:, (n % 2) * 512:
                                                              (n % 2 + 1) * 512],
                                         start=(k == 0), stop=(k == KH - 1))
                for n in range(NC1):
                    nc.scalar.copy(y1q[:, s, n * 512:(n + 1) * 512], ps[n][:])

        def mix1_fp8():
            # per-head seq mix in fp8 DoubleRow (256-deep per instruction),
            # then unscale+bias+square into feature-major f32r
            ysq = []
            for h in range(NH):
                pss = {}
                for dp in range(2):
                    for tc_i in range(2):
                        pss[(dp, tc_i)] = psum.tile(
                            [128, 512], f32, tag="mm", bufs=8,
                            name=f"m1p{h}_{dp}_{tc_i}")
                for sb in range(SB):
                    mt = pool.tile([128, 2, S], f8, tag="m1ring", bufs=8,
                                   name=f"m1t{h}_{sb}")
                    nc.sync.dma_start(mt[:], m1f8[h, sb])
                    for dp in range(2):
                        lhsT = y1q[:, 2 * sb:2 * sb + 2,
                                   h * DH + dp * 128:h * DH + (dp + 1) * 128]
                        for tc_i in range(2):
                            nc.tensor.matmul(
                                pss[(dp, tc_i)][:], lhsT,
                                mt[:, :, tc_i * 512:(tc_i + 1) * 512],
                                start=(sb == 0), stop=(sb == SB - 1),
                                perf_mode=PM.DoubleRow)
                for dp in range(2):
                    i = h * 2 + dp
                    yo = pool.tile([128, S], f32r, tag="featmaj", bufs=16,
                                   name=f"y1sq{i}")
                    for tc_i in range(2):
                        nc.scalar.activation(yo[:, tc_i * 512:(tc_i + 1) * 512],
                                             pss[(dp, tc_i)][:], AF.Square,
                                             bias=t_b1[:, i:i + 1],
                                             scale=t_c1[:, i:i + 1])
                    ysq.append(yo)
            return ysq

        def mix(yin, m_ap, bias_tile, out_name):
            # per-head seq mix + bias + square; token-major in, feature-major out
            ysq = []
            for h in range(NH):
                groups = [[None] * 2 for _ in range(2)]
                for tc_i in range(2):
                    for dp in range(2):
                        groups[tc_i][dp] = psum.tile(
                            [128, 512], f32, tag="mm", bufs=8,
                            name=f"{out_name}p{h}_{tc_i}_{dp}")
                for s in range(SC):
                    mt = pool.tile([128, S], bf16, tag="mring", bufs=8,
                                   name=f"{out_name}m{h}_{s}")
                    nc.sync.dma_start(mt[:], m_ap[h, s * 128:(s + 1) * 128, :])
                    for dp in range(2):
                        lhsT = yin[s][:, h * DH + dp * 128: h * DH + (dp + 1) * 128]
                        for tc_i in range(2):
                            nc.tensor.matmul(groups[tc_i][dp][:], lhsT,
                                             mt[:, tc_i * 512:(tc_i + 1) * 512],
                                             start=(s == 0), stop=(s == SC - 1))
                for dp in range(2):
                    i = h * 2 + dp
                    yo = pool.tile([128, S], f32r, tag="featmaj", bufs=16,
                                   name=f"{out_name}{i}")
                    for tc_i in range(2):
                        nc.scalar.activation(yo[:, tc_i * 512:(tc_i + 1) * 512],
                                             groups[tc_i][dp][:], AF.Square,
                                             bias=bias_tile[:, i:i + 1])
                    ysq.append(yo)
            return ysq

        def dense(xin, w_ap, nk, nm):
            # token-major out: y[s, n] = x @ w  (xin: feature-major f32r tiles)
            yt = []
            for s in range(SC):
                yt.append(pool.tile([128, INTER], bf16, tag="tokmaj", bufs=SC,
                                    name=f"{nm}{s}"))
            for n in range(NC1):
                wts = []
                for k in range(nk):
                    wt = pool.tile([128, 512], f32r, tag="wring", bufs=16,
                                   name=f"{nm}w{n}_{k}")
                    nc.sync.dma_start(wt[:], w_ap[k * 128:(k + 1) * 128,
                                                  n * 512:(n + 1) * 512]
                                      .bitcast(f32r))
                    wts.append(wt)
                for s in range(SC):
                    ps = psum.tile([128, 512], f32, tag="mm", bufs=8,
                                   name=f"{nm}p{n}_{s}")
                    for k in range(nk):
                        nc.tensor.matmul(ps[:], xin[k][:, s * 128:(s + 1) * 128],
                                         wts[k][:], start=(k == 0),
                                         stop=(k == nk - 1))
                    nc.scalar.copy(yt[s][:, n * 512:(n + 1) * 512], ps[:])
            return yt

        dense1()
        y1sq = mix1_fp8()
        if upto == 0:                       # stage C
            dump(y1sq)
        if upto >= 1:
            y2 = dense(y1sq, w2t, KI, "y2_")
            if upto == 1:
                dump(y2)
        if upto >= 2:
            y2sq = mix(y2, m2, t_b2, "y2sq")
            if upto == 2:
                dump(y2sq)
        if upto >= 3:                       # dense3 + bias + store
            w3ts = []
            for k in range(KI):
                w3tile = pool.tile([128, 512], f32r, tag="wring", bufs=16,
                                   name=f"w3_{k}")
                nc.sync.dma_start(w3tile[:],
                                  w3t[k * 128:(k + 1) * 128, :].bitcast(f32r))
                w3ts.append(w3tile)
            for ht in range(HT):
                yo = pool.tile([128, S], f32, tag="out", bufs=2, name=f"yt{ht}")
                pss = [psum.tile([128, 512], f32, tag="mm", bufs=8,
                                 name=f"p3_{ht}_{sc}") for sc in range(2)]
                for k in range(KI):
                    for sc in range(2):
                        nc.tensor.matmul(pss[sc][:],
                                         w3ts[k][:, ht * 128:(ht + 1) * 128],
                                         y2sq[k][:, sc * 512:(sc + 1) * 512],
                                         start=(k == 0), stop=(k == KI - 1))
                for sc in range(2):
                    nc.scalar.activation(yo[:, sc * 512:(sc + 1) * 512], pss[sc][:],
                                         AF.Identity, bias=t_b3[:, ht:ht + 1])
                    nc.sync.dma_start(
                        yt_out[ht * 128:(ht + 1) * 128, sc * 512:(sc + 1) * 512],
                        yo[:, sc * 512:(sc + 1) * 512])

    nc.compile()
    return nc


_PROGRAMS = {}
LAST_RESULT = None


def _get_program(stage="full"):
    if stage not in _PROGRAMS:
        _PROGRAMS[stage] = _build_program(stage)
    return _PROGRAMS[stage]


def _prep_maps(x, word_emb, pos_emb, tok_emb, emb_ln_w, emb_ln_b,
               init_d, init_b, init_M, inter0_d, inter0_b, inter0_M,
               final_d, final_b):
    x = np.asarray(x)
    f = lambda a: np.ascontiguousarray(np.asarray(a), dtype=np.float32)
    BF = ml_dtypes.bfloat16
    E4 = ml_dtypes.float8_e4m3

    # per-feature scales folded into w1 columns; safe wrt fp8e4 max 240:
    # |y1s| <= rownorm_bound * ||w1s_col|| = 192 * nudges (~205).
    # NUDGE_8/NUDGE_16: pre-scale values headed into a device-side bf16/fp8
    # cast by half a quantization bin so a truncating converter behaves like
    # round-to-nearest; divided back out in c1c so the algebra is exact.
    NUDGE_16 = 1.0 + 2.0 ** -8   # bf16 x0t cast (DVE/ACT write)
    NUDGE_8 = 1.0 + 2.0 ** -4    # fp8 y1 cast (ACT copy)
    w1 = f(init_d).T                                    # [HID, INTER]
    colnorm = np.linalg.norm(w1, axis=0)
    rown = np.sqrt(HID - 1.0) * np.abs(f(emb_ln_w)).max() \
        + np.linalg.norm(f(emb_ln_b))
    s_f = (192.0 / (rown * np.maximum(colnorm, 1e-20))).astype(np.float32)
    w1s = np.ascontiguousarray((w1 * (s_f * NUDGE_8)[None, :]).astype(BF))

    M1 = f(init_M)
    s_h = (192.0 / np.maximum(np.abs(M1).max(axis=(1, 2)), 1e-20)).astype(np.float32)
    m1q = np.ascontiguousarray(
        (M1 * s_h[:, None, None]).reshape(NH, SB, 2, 128, S)
        .transpose(0, 1, 3, 2, 4).astype(E4))
    c_full = (1.0 / (s_f * np.repeat(s_h, DH)
                     * NUDGE_8 * NUDGE_16)).astype(np.float32)

    shared = dict(
        word_emb=f(word_emb),
        posplus=f(pos_emb) + f(tok_emb)[0][None, :],
        lnw=np.ascontiguousarray(f(emb_ln_w).reshape(HT, 128).T) * np.float32(NUDGE_16),
        lnb=np.ascontiguousarray(f(emb_ln_b).reshape(HT, 128).T) * np.float32(NUDGE_16),
        w1t=w1s,
        b1c=np.ascontiguousarray(f(init_b).reshape(KI, 128).T),
        c1c=np.ascontiguousarray(c_full.reshape(KI, 128).T),
        m1f8=m1q,
        w2t=np.ascontiguousarray(f(inter0_d).T),
        b2c=np.ascontiguousarray(f(inter0_b).reshape(KI, 128).T),
        m2=np.ascontiguousarray(np.asarray(inter0_M)).astype(BF),
        w3t=np.ascontiguousarray(f(final_d).T),
        b3c=np.ascontiguousarray(f(final_b).reshape(HT, 128).T),
    )
    in_maps = []
    for b in range(B):
        xwb = np.ascontiguousarray(x[b].astype(np.int32).reshape(SC, 128).T)
        in_maps.append(dict(shared, xw=xwb))
    return in_maps


def kernel(**inputs):
    global LAST_RESULT
    stage = os.environ.get("KSTAGE", "full")
    ncores = int(os.environ.get("KCORES", str(N_CORES)))
    in_maps = _prep_maps(**inputs)[:ncores]
    nc = _get_program(stage)
    res = run_bass_kernel_spmd(nc, in_maps, list(range(ncores)))
    LAST_RESULT = res
    out = np.stack([res.results[b]["yt"].T for b in range(ncores)])
    if ncores < B:
        out = np.concatenate([out] + [out[:1]] * (B - ncores))
    return out


# revision 13
# speedup vs baseline: 1.0950x; 1.0232x over previous
"""BertTinyFlatten on 8 Trainium2 NeuronCores — data-parallel over batch.

Per core (one batch element):
  emb   = gather(word_emb, x) + (pos_emb + tok_emb[0])      [bf16, indirect DMA w/ CCE add]
  x0    = layernorm(emb)                                     [bf16 out]
  x0t   = x0.T (PE transpose bf16, LN affine fused into PSUM->SBUF copy)
  y1q   = fp8e4(x0 @ w1_scaled)  (k-outer bf16 matmuls; per-feature scales
          folded into w1 columns host-side; ACT copy casts PSUM->fp8)
  y1sq  = (mix_fp8dr(y1q, M1q) * c + b1)^2 -> feature-major  [fp8 DoubleRow
          matmuls at 2x rate; scales unfolded via ACT Square scale/bias]
  y2    = y1sq-chain @ inter0_d.T     -> token-major bf16    [f32r matmuls]
  y2sq  = (mix(y2, inter0_M) + b2)^2  -> feature-major       [bf16 matmuls]
  yt    = final_d-chain @ y2sq + b3   -> feature-major; host transposes back

DMA plan: w1 halfslabs interleaved with posplus chunks on the sync queue so
dense1 can start ~13us; gathers (software DGE) issue immediately after t_idx.
"""
import os
import sys

import numpy as np
import ml_dtypes

for _p in ("/opt/trn_rl_repo", "/opt/pypackages"):
    if _p not in sys.path and os.path.isdir(_p):
        sys.path.append(_p)

from contextlib import ExitStack

import concourse.bass as bass
import concourse.tile as tile
from concourse import bacc, masks, mybir
from concourse.bass import IndirectOffsetOnAxis
from concourse.bass_utils import run_bass_kernel_spmd

f32 = mybir.dt.float32
f32r = mybir.dt.float32r
bf16 = mybir.dt.bfloat16
f8 = mybir.dt.float8e4
i32 = mybir.dt.int32
AF = mybir.ActivationFunctionType
ALU = mybir.AluOpType
AX = mybir.AxisListType
PM = mybir.MatmulPerfMode

B, S, HID, NH, INTER, VOCAB = 8, 1024, 512, 8, 2048, 30522
DH = INTER // NH            # 256 features per head
EPS = 1e-12
N_CORES = 8

KH = HID // 128             # 4   k-tiles for dense1
KI = INTER // 128           # 16  k-tiles for dense2/3
SC = S // 128               # 8   token chunks
NC1 = INTER // 512          # 4   n-chunks (512) for dense1/2
HT = HID // 128             # 4   hid tiles for dense3
SB = S // 256               # 4   256-deep s-blocks for fp8 DoubleRow mix

STAGES = ("C", "D", "E", "full")


def _build_program(stage="full"):
    upto = STAGES.index(stage)
    nc = bacc.Bacc("TRN2", target_bir_lowering=False, debug=False,
                   num_devices=N_CORES, num_swdge_queues=2)

    xw = nc.dram_tensor("xw", [128, SC], i32, kind="ExternalInput").ap()
    word_emb = nc.dram_tensor("word_emb", [VOCAB, HID], f32, kind="ExternalInput").ap()
    posplus = nc.dram_tensor("posplus", [S, HID], f32, kind="ExternalInput").ap()
    lnw = nc.dram_tensor("lnw", [128, HT], f32, kind="ExternalInput").ap()
    lnb = nc.dram_tensor("lnb", [128, HT], f32, kind="ExternalInput").ap()
    w1t = nc.dram_tensor("w1t", [HID, INTER], bf16, kind="ExternalInput").ap()
    b1c = nc.dram_tensor("b1c", [128, KI], f32, kind="ExternalInput").ap()
    c1c = nc.dram_tensor("c1c", [128, KI], f32, kind="ExternalInput").ap()
    m1f8 = nc.dram_tensor("m1f8", [NH, SB, 128, 2, S], f8, kind="ExternalInput").ap()
    w2t = nc.dram_tensor("w2t", [INTER, INTER], f32, kind="ExternalInput").ap()
    b2c = nc.dram_tensor("b2c", [128, KI], f32, kind="ExternalInput").ap()
    m2 = nc.dram_tensor("m2", [NH, S, S], bf16, kind="ExternalInput").ap()
    w3t = nc.dram_tensor("w3t", [INTER, HID], f32, kind="ExternalInput").ap()
    b3c = nc.dram_tensor("b3c", [128, HT], f32, kind="ExternalInput").ap()
    yt_out = nc.dram_tensor("yt", [HID, S], f32, kind="ExternalOutput").ap()

    with tile.TileContext(nc) as tc, ExitStack() as ctx:
        pool = ctx.enter_context(tc.tile_pool(name="sbuf", bufs=1))
        psum = ctx.enter_context(tc.tile_pool(name="psum", bufs=1, space="PSUM"))

        def dump(tiles):
            # debug: write up to four [128, >=S]-byte tiles (as f32 words)
            for i, t in enumerate(tiles[:4]):
                v = t[:].bitcast(f32)
                w = min(v.shape[-1], S)
                nc.sync.dma_start(yt_out[i * 128:(i + 1) * 128, 0:w], v[:, 0:w])

        # ---- token ids first: the gathers depend only on this ----------
        t_idx = pool.tile([128, SC], i32)
        nc.sync.dma_start(t_idx[:], xw[:])

        # ---- constants needed by the PE early: emit on gpsimd BEFORE the
        # serialized gather instructions.
        ident = pool.tile([128, 128], f32)
        masks.make_identity(nc, ident[:])
        zerocol = pool.tile([128, 1], f32)
        nc.vector.memset(zerocol[:], 0.0)

        # ---- posplus chunks interleaved with w1 halfslabs on sync:
        # chunk c first so gather c can start, w1 k-slabs in PE use order.
        emb = []
        w1hs = []
        for c in range(SC):
            e = pool.tile([128, HID], f32, tag="emb", bufs=SC, name=f"emb{c}")
            nc.sync.dma_start(e[:], posplus[c * 128:(c + 1) * 128, :])
            emb.append(e)
            for j in ([0, 1] if c == 0 else [c * 2, c * 2 + 1]) if c < KH else []:
                w = pool.tile([128, 1024], bf16, tag="wring", bufs=24,
                              name=f"w1hs{j}")
                nc.sync.dma_start(w[:], w1t[(j // 2) * 128:(j // 2 + 1) * 128,
                                            (j % 2) * 1024:(j % 2 + 1) * 1024])
                w1hs.append(w)

        # ---- indirect gathers add word_emb rows onto posplus chunks ----
        for c in range(SC):
            gi = nc.gpsimd.indirect_dma_start(
                out=emb[c][:], out_offset=None,
                in_=word_emb[:128, :],
                in_offset=IndirectOffsetOnAxis(ap=t_idx[:, c:c + 1], axis=0),
                bounds_check=VOCAB - 1, oob_is_err=False,
                compute_op=ALU.add,
            )
            if c % 2:
                gi.ins.queue = "qPoolDynamic1"

        # ---- remaining constants ---------------------------------------
        t_lnw = pool.tile([128, HT], f32)
        nc.sync.dma_start(t_lnw[:], lnw[:])
        t_lnb = pool.tile([128, HT], f32)
        nc.sync.dma_start(t_lnb[:], lnb[:])
        t_b1 = pool.tile([128, KI], f32)
        nc.sync.dma_start(t_b1[:], b1c[:])
        t_c1 = pool.tile([128, KI], f32)
        nc.sync.dma_start(t_c1[:], c1c[:])
        t_b2 = pool.tile([128, KI], f32)
        nc.sync.dma_start(t_b2[:], b2c[:])
        t_b3 = pool.tile([128, HT], f32)
        nc.sync.dma_start(t_b3[:], b3c[:])

        def layernorm_chunk(c):
            # all-DVE except the Square+accum (Scalar, runs concurrently):
            # rstd via fast-inverse-sqrt (2 Newton iters, rel err < 5e-6)
            # to avoid Vector<->Scalar semaphore round trips on the head
            # critical path.
            e = emb[c]
            msum = pool.tile([128, 1], f32, tag="msum", bufs=2, name=f"msum{c}")
            nc.vector.reduce_sum(msum[:], e[:], axis=AX.X)
            sqd = pool.tile([128, HID], f32, tag="tokmaj", bufs=SC, name=f"sqd{c}")
            ssq = pool.tile([128, 1], f32, tag="ssq", bufs=2, name=f"ssq{c}")
            nc.scalar.activation(sqd[:], e[:], AF.Square, bias=zerocol[:],
                                 accum_out=ssq[:])
            # ve = var + eps = ssq/H - (msum^2/H^2 - eps)
            t1 = pool.tile([128, 1], f32, tag="t1", bufs=2, name=f"t1_{c}")
            nc.vector.tensor_scalar(t1[:], msum[:], msum[:], 1.0 / (HID * HID),
                                    op0=ALU.mult, op1=ALU.mult)
            nc.vector.tensor_scalar(t1[:], t1[:], EPS, None, op0=ALU.subtract)
            ve = pool.tile([128, 1], f32, tag="var", bufs=2, name=f"var{c}")
            nc.vector.tensor_scalar(ve[:], ssq[:], 1.0 / HID, t1[:],
                                    op0=ALU.mult, op1=ALU.subtract)
            # r0 = bitcast(0x5F3759DF - (bitcast_i32(ve) >> 1))
            rstd = pool.tile([128, 1], f32, tag="rstd", bufs=2, name=f"rstd{c}")
            ri = rstd[:].bitcast(i32)
            nc.vector.tensor_scalar(ri, ve[:].bitcast(i32), 1, None,
                                    op0=ALU.arith_shift_right)
            nc.vector.tensor_scalar(ri, ri, -1, 0x5F3759DF,
                                    op0=ALU.mult, op1=ALU.add)
            # Newton x2: r = r * (1.5 - 0.5*ve*r^2)
            nt = pool.tile([128, 1], f32, tag="nt", bufs=2, name=f"nt{c}")
            for _ in range(2):
                nc.vector.tensor_scalar(nt[:], rstd[:], rstd[:], ve[:],
                                        op0=ALU.mult, op1=ALU.mult)
                nc.vector.tensor_scalar(nt[:], nt[:], -0.5, 1.5,
                                        op0=ALU.mult, op1=ALU.add)
                nc.vector.tensor_scalar(rstd[:], rstd[:], nt[:], None,
                                        op0=ALU.mult)
            # e = e*rstd - (msum/H)*rstd
            mr = pool.tile([128, 1], f32, tag="mr", bufs=2, name=f"mr{c}")
            nc.vector.tensor_scalar(mr[:], msum[:], rstd[:], 1.0 / HID,
                                    op0=ALU.mult, op1=ALU.mult)
            nc.vector.tensor_scalar(e[:], e[:], rstd[:], mr[:],
                                    op0=ALU.mult, op1=ALU.subtract)

        # feature-major f32r ring: x0t slots (bitcast bf16) -> y1sq (16)
        # -> y2sq (16), WAR-serialized by Tile.
        x0t = []
        for ht in range(HT):
            slot = pool.tile([128, S], f32r, tag="featmaj", bufs=16,
                             name=f"x0t{ht}")
            x0t.append(slot[:].bitcast(bf16))           # [128, 2S] bf16 view

        # y1 in fp8, laid out [128 tok-in-chunk, SC, INTER]
        y1q = pool.tile([128, SC, INTER], f8, name="y1q")

        def dense1():
            # k-outer so early w1 slabs are consumed as they arrive
            for s in range(SC):
                layernorm_chunk(s)
                for ht in range(HT):
                    pt = psum.tile([128, 512], f32, tag="mm", bufs=8,
                                   name=f"ptr{ht}_{s}")
                    nc.tensor.transpose(pt[:, 0:128],
                                        emb[s][:, ht * 128:(ht + 1) * 128],
                                        ident[:])
                    nc.vector.tensor_scalar(
                        x0t[ht][:, s * 128:(s + 1) * 128], pt[:, 0:128],
                        t_lnw[:, ht:ht + 1], t_lnb[:, ht:ht + 1],
                        op0=ALU.mult, op1=ALU.add)
                ps = [psum.tile([128, 512], f32, tag="mm", bufs=8,
                                name=f"d1p{n}_{s}") for n in range(NC1)]
                for k in range(KH):
                    for n in range(NC1):
                        nc.tensor.matmul(ps[n][:],
                                         x0t[k][:, s * 128:(s + 1) * 128],
                                         w1hs[k * 2 + n // 2][:, (n % 2) * 512:
                                                              (n % 2 + 1) * 512],
                                         start=(k == 0), stop=(k == KH - 1))
                for n in range(NC1):
                    nc.scalar.copy(y1q[:, s, n * 512:(n + 1) * 512], ps[n][:])

        def mix1_fp8():
            # per-head seq mix in fp8 DoubleRow (256-deep per instruction),
            # then unscale+bias+square into feature-major f32r
            ysq = []
            for h in range(NH):
                pss = {}
                for dp in range(2):
                    for tc_i in range(2):
                        pss[(dp, tc_i)] = psum.tile(
                            [128, 512], f32, tag="mm", bufs=8,
                            name=f"m1p{h}_{dp}_{tc_i}")
                for sb in range(SB):
                    mt = pool.tile([128, 2, S], f8, tag="emb", bufs=SC,
                                   name=f"m1t{h}_{sb}")
                    nc.sync.dma_start(mt[:], m1f8[h, sb])
                    for dp in range(2):
                        lhsT = y1q[:, 2 * sb:2 * sb + 2,
                                   h * DH + dp * 128:h * DH + (dp + 1) * 128]
                        for tc_i in range(2):
                            nc.tensor.matmul(
                                pss[(dp, tc_i)][:], lhsT,
                                mt[:, :, tc_i * 512:(tc_i + 1) * 512],
                                start=(sb == 0), stop=(sb == SB - 1),
                                perf_mode=PM.DoubleRow)
                for dp in range(2):
                    i = h * 2 + dp
                    yo = pool.tile([128, S], f32r, tag="featmaj", bufs=16,
                                   name=f"y1sq{i}")
                    for tc_i in range(2):
                        nc.scalar.activation(yo[:, tc_i * 512:(tc_i + 1) * 512],
                                             pss[(dp, tc_i)][:], AF.Square,
                                             bias=t_b1[:, i:i + 1],
                                             scale=t_c1[:, i:i + 1])
                    ysq.append(yo)
            return ysq

        def mix(yin, m_ap, bias_tile, out_name):
            # per-head seq mix + bias + square; token-major in, feature-major out
            ysq = []
            for h in range(NH):
                groups = [[None] * 2 for _ in range(2)]
                for tc_i in range(2):
                    for dp in range(2):
                        groups[tc_i][dp] = psum.tile(
                            [128, 512], f32, tag="mm", bufs=8,
                            name=f"{out_name}p{h}_{tc_i}_{dp}")
                for s in range(SC):
                    mt = pool.tile([128, S], bf16, tag="emb", bufs=SC,
                                   name=f"{out_name}m{h}_{s}")
                    nc.sync.dma_start(mt[:], m_ap[h, s * 128:(s + 1) * 128, :])
                    for dp in range(2):
                        lhsT = yin[s][:, h * DH + dp * 128: h * DH + (dp + 1) * 128]
                        for tc_i in range(2):
                            nc.tensor.matmul(groups[tc_i][dp][:], lhsT,
                                             mt[:, tc_i * 512:(tc_i + 1) * 512],
                                             start=(s == 0), stop=(s == SC - 1))
                for dp in range(2):
                    i = h * 2 + dp
                    yo = pool.tile([128, S], f32r, tag="featmaj", bufs=16,
                                   name=f"{out_name}{i}")
                    for tc_i in range(2):
                        nc.scalar.activation(yo[:, tc_i * 512:(tc_i + 1) * 512],
                                             groups[tc_i][dp][:], AF.Square,
                                             bias=bias_tile[:, i:i + 1])
                    ysq.append(yo)
            return ysq

        def dense(xin, w_ap, nk, nm):
            # token-major out: y[s, n] = x @ w  (xin: feature-major f32r tiles)
            yt = []
            for s in range(SC):
                yt.append(pool.tile([128, INTER], bf16, tag="tokmaj", bufs=SC,
                                    name=f"{nm}{s}"))
            for n in range(NC1):
                wts = []
                for k in range(nk):
                    wt = pool.tile([128, 512], f32r, tag="wring", bufs=24,
                                   name=f"{nm}w{n}_{k}")
                    nc.sync.dma_start(wt[:], w_ap[k * 128:(k + 1) * 128,
                                                  n * 512:(n + 1) * 512]
                                      .bitcast(f32r))
                    wts.append(wt)
                for s in range(SC):
                    ps = psum.tile([128, 512], f32, tag="mm", bufs=8,
                                   name=f"{nm}p{n}_{s}")
                    for k in range(nk):
                        nc.tensor.matmul(ps[:], xin[k][:, s * 128:(s + 1) * 128],
                                         wts[k][:], start=(k == 0),
                                         stop=(k == nk - 1))
                    nc.scalar.copy(yt[s][:, n * 512:(n + 1) * 512], ps[:])
            return yt

        dense1()
        y1sq = mix1_fp8()
        if upto == 0:                       # stage C
            dump(y1sq)
        if upto >= 1:
            y2 = dense(y1sq, w2t, KI, "y2_")
            if upto == 1:
                dump(y2)
        if upto >= 2:
            y2sq = mix(y2, m2, t_b2, "y2sq")
            if upto == 2:
                dump(y2sq)
        if upto >= 3:                       # dense3 + bias + store
            w3ts = []
            for k in range(KI):
                w3tile = pool.tile([128, 512], f32r, tag="wring", bufs=24,
                                   name=f"w3_{k}")
                nc.sync.dma_start(w3tile[:],
                                  w3t[k * 128:(k + 1) * 128, :].bitcast(f32r))
                w3ts.append(w3tile)
            for ht in range(HT):
                yo = pool.tile([128, S], f32, tag="out", bufs=2, name=f"yt{ht}")
                pss = [psum.tile([128, 512], f32, tag="mm", bufs=8,
                                 name=f"p3_{ht}_{sc}") for sc in range(2)]
                for k in range(KI):
                    for sc in range(2):
                        nc.tensor.matmul(pss[sc][:],
                                         w3ts[k][:, ht * 128:(ht + 1) * 128],
                                         y2sq[k][:, sc * 512:(sc + 1) * 512],
                                         start=(k == 0), stop=(k == KI - 1))
                for sc in range(2):
                    nc.scalar.activation(yo[:, sc * 512:(sc + 1) * 512], pss[sc][:],
                                         AF.Identity, bias=t_b3[:, ht:ht + 1])
                    nc.sync.dma_start(
                        yt_out[ht * 128:(ht + 1) * 128, sc * 512:(sc + 1) * 512],
                        yo[:, sc * 512:(sc + 1) * 512])

    nc.compile()
    return nc


_PROGRAMS = {}
LAST_RESULT = None


def _get_program(stage="full"):
    if stage not in _PROGRAMS:
        _PROGRAMS[stage] = _build_program(stage)
    return _PROGRAMS[stage]


def _prep_maps(x, word_emb, pos_emb, tok_emb, emb_ln_w, emb_ln_b,
               init_d, init_b, init_M, inter0_d, inter0_b, inter0_M,
               final_d, final_b):
    x = np.asarray(x)
    f = lambda a: np.ascontiguousarray(np.asarray(a), dtype=np.float32)
    BF = ml_dtypes.bfloat16
    E4 = ml_dtypes.float8_e4m3

    # per-feature scales folded into w1 columns; safe wrt fp8e4 max 240:
    # |y1s| <= rownorm_bound * ||w1s_col|| = 192 * nudges (~205).
    # NUDGE_8/NUDGE_16: pre-scale values headed into a device-side bf16/fp8
    # cast by half a quantization bin so a truncating converter behaves like
    # round-to-nearest; divided back out in c1c so the algebra is exact.
    NUDGE_16 = 1.0 + 2.0 ** -8   # bf16 x0t cast (DVE/ACT write)
    NUDGE_8 = 1.0 + 2.0 ** -4    # fp8 y1 cast (ACT copy)
    w1 = f(init_d).T                                    # [HID, INTER]
    colnorm = np.linalg.norm(w1, axis=0)
    rown = np.sqrt(HID - 1.0) * np.abs(f(emb_ln_w)).max() \
        + np.linalg.norm(f(emb_ln_b))
    s_f = (192.0 / (rown * np.maximum(colnorm, 1e-20))).astype(np.float32)
    w1s = np.ascontiguousarray((w1 * (s_f * NUDGE_8)[None, :]).astype(BF))

    M1 = f(init_M)
    s_h = (192.0 / np.maximum(np.abs(M1).max(axis=(1, 2)), 1e-20)).astype(np.float32)
    m1q = np.ascontiguousarray(
        (M1 * s_h[:, None, None]).reshape(NH, SB, 2, 128, S)
        .transpose(0, 1, 3, 2, 4).astype(E4))
    c_full = (1.0 / (s_f * np.repeat(s_h, DH)
                     * NUDGE_8 * NUDGE_16)).astype(np.float32)

    shared = dict(
        word_emb=f(word_emb),
        posplus=f(pos_emb) + f(tok_emb)[0][None, :],
        lnw=np.ascontiguousarray(f(emb_ln_w).reshape(HT, 128).T) * np.float32(NUDGE_16),
        lnb=np.ascontiguousarray(f(emb_ln_b).reshape(HT, 128).T) * np.float32(NUDGE_16),
        w1t=w1s,
        b1c=np.ascontiguousarray(f(init_b).reshape(KI, 128).T),
        c1c=np.ascontiguousarray(c_full.reshape(KI, 128).T),
        m1f8=m1q,
        w2t=np.ascontiguousarray(f(inter0_d).T),
        b2c=np.ascontiguousarray(f(inter0_b).reshape(KI, 128).T),
        m2=np.ascontiguousarray(np.asarray(inter0_M)).astype(BF),
        w3t=np.ascontiguousarray(f(final_d).T),
        b3c=np.ascontiguousarray(f(final_b).reshape(HT, 128).T),
    )
    in_maps = []
    for b in range(B):
        xwb = np.ascontiguousarray(x[b].astype(np.int32).reshape(SC, 128).T)
        in_maps.append(dict(shared, xw=xwb))
    return in_maps


def kernel(**inputs):
    global LAST_RESULT
    stage = os.environ.get("KSTAGE", "full")
    ncores = int(os.environ.get("KCORES", str(N_CORES)))
    in_maps = _prep_maps(**inputs)[:ncores]
    nc = _get_program(stage)
    res = run_bass_kernel_spmd(nc, in_maps, list(range(ncores)))
    LAST_RESULT = res
    out = np.stack([res.results[b]["yt"].T for b in range(ncores)])
    if ncores < B:
        out = np.concatenate([out] + [out[:1]] * (B - ncores))
    return out
